# revision 39
# baseline (speedup 1.0000x reference)
"""DigitCaps dynamic-routing kernel for 8 TRN2 NeuronCores.

Algorithm (never materializes u_hat):
  Shard over capsules C=96 -> 12 per core (makes every routing step
  core-local: softmax over R is per-capsule, the batch-mean a_ij needs
  no cross-core reduction -> zero collectives).

  Per core, with K = R*I = 3840 the flattened contraction dim:

  s-phase (k-layout A: chunk t <-> i=t//2, r=(t%2)*96+p):
    s[b,(c,o)]    = sum_k wc[k,(c,o)] * xt[k, b]           (PE, 80 mm N=192)
    wc[k,(c,o)]   = cB[k,(c,o)] * wt[k,(c,o)]              (DVE bf16 2x)
    v = squash(s)                                          (ACT+DVE, small)

  a-phase (k-layout B: k = r*20 + i, (c,o) on partitions):
    M'[(c,o), k]  = sum_b v[b,(c,o)] * x[b, k]             (PE, 32 mm N=480,
                                                            stationary=v reused)
    P = W2 .* M'  (ACT copies psum->bf16, DVE 2x muls)
    Q[(c,o), r]   = sum_i P   (g=0 on DVE, g=1 on GpSimd -- parallel)
    a[c, r]       = SEL.T @ Q  (SEL = delta(c)/B)          (PE, tiny)
    b_ij += a; c_ij = softmax_r(b_ij)  (two [6,192] half-tiles)

  PE HAM management: a startup chain of 1x1 matmuls (anchored on the
  inline ident constant) keeps the PE busy through the input-DMA wait so
  the clock is at 2.4 GHz when the real matmuls start; keep-warm anchors
  are woven through the routing math so the HAM MID window never sees a
  fully idle 3.4us stretch.

  Matmul inputs bf16 (measured ~5e-3 global rel err vs f32 reference),
  routing/softmax/squash math in f32.
"""

import numpy as np
import ml_dtypes

import concourse.bass as bass
import concourse.mybir as mybir
from concourse import tile
from concourse.vector_clock import ScopedClock

B, R, C, O, I = 256, 192, 96, 16, 20
NCORES = 8
CL = C // NCORES          # 12 capsules per core
M = CL * O                # 192 = (c,o) free dim per core
KC = R // 2               # 96 = k-chunk size for s-phase (partition dim)
NK = 2 * I                # 40 s-phase k-chunks
KT = R * I                # 3840 total contraction
NJ = 8                    # a-phase moving slices (3840 = 8 * 480)
JW = KT // NJ             # 480
NITER = 3
NW = 5                    # input DMA split for wt/xt (8 chunks per DMA)
CPW = NK // NW            # 8
KH = KT // 2              # 1920 = xb/w2 DMA half

F32 = mybir.dt.float32
BF16 = mybir.dt.bfloat16
AF = mybir.ActivationFunctionType
ALU = mybir.AluOpType
AX = mybir.AxisListType


class _TC(tile.TileContext):
    """TileContext whose exit drain splits its semaphore waits across
    chained SP nops -- the walrus in this container caps sync-waits per
    CTRL instruction at 1."""

    def _drain_and_barrier(self, tick_clock, wait_clock):
        nc = self.nc
        lead = nc.sync.nop(nofuse=True)
        wait_clock.add_sem_waits(
            lead.ins, ScopedClock({None: tick_clock.global_clock})
        )
        si = lead.ins.sync_info
        waits = list(si.on_wait) if (si and si.on_wait) else []
        if len(waits) > 1:
            si.on_wait = waits[:1]
            # distribute the remaining waits round-robin across all engine
            # sequencers -- they run in parallel and the all_engine_barrier
            # below joins them, so this is ~5x faster than a serial SP chain
            engs = [nc.sync, nc.vector, nc.scalar, nc.tensor, nc.gpsimd]
            for k, w in enumerate(waits[1:]):
                n = engs[k % len(engs)].nop(nofuse=True)
                nsi = n.ins.sync_info
                if nsi is None:
                    n.ins.sync_info = mybir.SyncInfo(on_wait=[w], on_update=[])
                else:
                    nsi.on_wait = [w]
        nc.sync.drain()
        nc.all_engine_barrier()
        assert self.sems is not None
        popped = nc._tile_sem_poison_stack.pop()
        assert popped is self._sem_poison
        # final barrier elided: the clears run on gpsimd's stream and
        # end-of-execution engine completion already covers them
        nc.clear_and_free_semaphores(list(self.sems.allocated().values()))


def _split_multi_waits(nc):
    """The walrus build in this container caps sync-waits at 1 per
    instruction. Hoist extra waits onto same-engine nops inserted just
    before the offending instruction (engine sequencers are serial, so
    chained single-wait nops are semantically identical)."""
    cur = nc.cur_bb.bb

    def make_nop(engine):
        bi = nc.engines[engine].nop(nofuse=True)
        lst = cur.instructions
        assert lst[-1].name == bi.ins.name
        cur.instructions = lst[:-1]
        return bi.ins

    for f in nc.m.functions:
        for bb in f.blocks:
            insts = bb.instructions
            out = []
            changed = False
            for ins in insts:
                si = ins.sync_info
                waits = list(si.on_wait) if (si and si.on_wait) else []
                if len(waits) > 1:
                    changed = True
                    for w in waits[:-1]:
                        nop = make_nop(ins.engine)
                        nsi = nop.sync_info
                        if nsi is None:
                            nop.sync_info = mybir.SyncInfo(
                                on_wait=[w], on_update=[]
                            )
                        else:
                            nsi.on_wait = [w]
                        out.append(nop)
                    si.on_wait = waits[-1:]
                out.append(ins)
            if changed:
                bb.instructions = out
    return nc


def _sel_const():
    # SEL[p, j] = 1/B where p//16 == j: contracts o within a (c,o)-group
    # of 96 partitions down to 6 capsules, folding the batch-mean scale.
    sel = np.zeros((KC, CL // 2), dtype=ml_dtypes.bfloat16)
    for p in range(KC):
        sel[p, p // O] = 1.0 / B
    return sel


def build_nc():
    nc = bass.Bass()
    # wt and xt fused along the free axis: one DMA per tile delivers both
    wx_d = nc.declare_dram_parameter("wx", [NW, KC, CPW, M + B], BF16, isOutput=False)
    xb_d = nc.declare_dram_parameter("xb", [128, 2, KT], BF16, isOutput=False)
    w2t_d = nc.declare_dram_parameter("w2t", [128, 30, M], BF16, isOutput=False)
    selc_d = nc.declare_dram_parameter("selc", [128, 3, R], BF16, isOutput=False)
    out_d = nc.declare_dram_parameter("out", [B, M], F32, isOutput=True)
    repl = np.zeros((CL // 2, KC), dtype=np.float32)
    for j in range(KC):
        repl[j // O, j] = 1.0
    repl_d = nc.inline_tensor(repl, "repl")
    sel_d = nc.inline_tensor(_sel_const(), "sel")

    with _TC(nc) as tc:
        with (
            tc.tile_pool(name="big", bufs=1) as big,
            tc.tile_pool(name="wcp", bufs=1) as wcp,
            tc.tile_pool(name="sm", bufs=2) as sm,
            tc.tile_pool(name="ps_s", bufs=1, space="PSUM") as ps_s,
            tc.tile_pool(name="ps_m", bufs=2, space="PSUM") as ps_m,
            tc.tile_pool(name="ps_t", bufs=1, space="PSUM") as ps_t,
            tc.tile_pool(name="ps_q", bufs=1, space="PSUM") as ps_q,
        ):
            # ---- persistent SBUF tensors -------------------------------
            wx_t = [big.tile([KC, CPW, M + B], BF16, tag=f"wx{j}", name=f"wx{j}")
                    for j in range(NW)]
            # xb / w2 in two k-halves each so the a-phase can start after
            # half the transfer has landed
            xb_t = [big.tile([128, 2, KH], BF16, tag=f"xb{h}", name=f"xb{h}")
                    for h in range(2)]
            w2t_t = big.tile([128, 30, M], BF16, tag="w2t", name="w2t")
            selc_t = big.tile([128, 3, R], BF16, tag="selc", name="selc")
            repl_t = big.tile([CL // 2, KC], F32, tag="repl")
            sel = big.tile([KC, CL // 2], BF16, tag="sel")
            bT = [big.tile([CL // 2, R], F32, tag=f"bT{h}", name=f"bT{h}")
                  for h in range(2)]

            # One HWDGE ring (sync), FIFO = consumption order: the tiny
            # constants jump the queue, then wx (gates the s-matmuls),
            # then xb/w2 halves interleaved in a-phase need order.
            nc.sync.dma_start(repl_t[:], repl_d[:])
            nc.sync.dma_start(sel[:], sel_d[:])
            for j in range(NW):
                # two transfers per chunk: tile deps are range-based, so
                # the first 4 sub-chunks of matmuls release after half
                # the bytes
                nc.sync.dma_start(wx_t[j][:, 0:CPW // 2, :],
                                  wx_d[j, :, 0:CPW // 2, :])
                nc.sync.dma_start(wx_t[j][:, CPW // 2:CPW, :],
                                  wx_d[j, :, CPW // 2:CPW, :])
            nc.sync.dma_start(xb_t[0][:], xb_d[:, :, 0:KH])
            nc.sync.dma_start(w2t_t[:, 0:15, :], w2t_d[:, 0:15, :])
            nc.sync.dma_start(xb_t[1][:], xb_d[:, :, KH:KT])
            nc.sync.dma_start(w2t_t[:, 15:30, :], w2t_d[:, 15:30, :])
            nc.sync.dma_start(selc_t[:], selc_d[:])

            def wt_c(t):   # wt chunk t -> [96, 192] AP
                return wx_t[t // CPW][:, t % CPW, 0:M]

            def xt_c(t, bt):  # [96, 128] lhsT for s-matmul
                return wx_t[t // CPW][:, t % CPW, M + bt * 128:M + (bt + 1) * 128]

            # wc lives in one chunked pool (bf16), full (c,o) width
            wc_t = [wcp.tile([KC, CPW, M], BF16, tag=f"wc{j}", name=f"wc{j}")
                    for j in range(NW)]

            def wc_c(t):
                return wc_t[t // CPW][:, t % CPW, :]

            co = dict(o=O)

            # Warm tile: HAM watches PE-*array* activity, so keep-warm
            # matmuls must be real-sized (128-partition stationary, wide
            # moving) -- 1x1 matmuls do not register. W0 is memset (no
            # DMA dependency).
            w0 = big.tile([128, 512], BF16, tag="warm", name="warm0")
            w0f = big.tile([128, 128], F32, tag="warmf", name="warmf")
            nc.gpsimd.memset(w0[:], 0.0)
            nc.gpsimd.memset(w0f[:], 0.0)

            def warm(anchor, n=2):
                # dummy matmuls that keep the PE array busy (HAM watches
                # array duty) through DVE/ACT stretches.  The first MM of
                # a burst reads `anchor` as its moving operand, inheriting
                # the routing chain's data dependency (placement inside
                # the stall window); the rest cycle through the quarters
                # of the warm psum tile so the WAW semaphore round-trip
                # is only paid every 4th MM.
                p = anchor.shape[0]
                w = min(anchor.shape[-1], 512)
                lhs = (w0 if anchor.dtype == BF16 else w0f)[0:p, 0:128]
                wp = ps_t.tile([128, 512], F32, tag="pst", name="warmp")
                nc.tensor.matmul(
                    wp[0:128, 0:w], lhs, anchor[:, 0:w], start=True, stop=True
                )
                for i in range(n - 1):
                    wp = ps_t.tile([128, 512], F32, tag="pst", name="warmp")
                    q = i % 4
                    nc.tensor.matmul(
                        wp[:, q * 128:(q + 1) * 128], w0[:, 0:128],
                        w0[:, 0:128], start=True, stop=True
                    )
            # ---- startup warm-up: PE-array busy from ~5us (preamble
            # end) until the first wx chunk arrives (~12us) so the HAM
            # un-throttles before the real matmuls start.
            for _ in range(18):
                wp = ps_t.tile([128, 512], F32, tag="pst", name="warmp")
                nc.tensor.matmul(
                    wp[:], w0[:, 0:128], w0[:], start=True, stop=True
                )

            vT = None
            for it in range(NITER):
                last = it == NITER - 1

                # ---- s-matmul: s[b,(c,o)] accumulated over 40 chunks,
                # full 192-wide MMs; bt=0 fully first so its squash ops
                # overlap bt=1's matmuls.
                s_ps = [ps_s.tile([128, M], F32, tag=f"s{bt}", name=f"s{bt}")[:]
                        for bt in range(2)]
                src = wt_c if it == 0 else wc_c
                for bt in range(2):
                    for t in range(NK):
                        nc.tensor.matmul(
                            s_ps[bt],
                            xt_c(t, bt),
                            src(t),
                            start=(t == 0),
                            stop=(t == NK - 1),
                        )
                        if it == 0 and t % (CPW // 2) == CPW // 2 - 1:
                            # it0 is DMA-paced (each wx sub-chunk arrives
                            # ~0.6us after the PE finishes the previous
                            # one) -- keep the HAM window non-idle
                            # through the arrival gaps.
                            warm(wx_t[t // CPW][:, t % CPW, :], n=2)

                # ---- squash: consolidated (per-bt wide ops + joint
                # small ops on [128, 2*CL]) ---------------------------
                scale = 1.0 / R if it == 0 else 1.0
                vT = sm.tile([128, 2, M], BF16, tag="vT")
                vOut = (
                    sm.tile([128, 2, M], F32, tag="vOut", name="vOut")
                    if last
                    else None
                )
                n2 = sm.tile([128, 2, CL], F32, tag="n2")
                for bt in range(2):
                    sq = sm.tile([128, M], F32, tag="sq", bufs=2)
                    nc.scalar.activation(sq[:], s_ps[bt], AF.Square, scale=scale)
                    nc.vector.reduce_sum(
                        n2[:, bt, :], sq[:].rearrange("p (c o) -> p c o", **co),
                        axis=AX.X,
                    )
                # keep-warm through the squash small-op chain (emitted
                # after both bt matmul groups so it can't delay them);
                # anchor = the just-consumed final weight chunk (wide and
                # complete, so the burst starts right here in FIFO order)
                wsrc = wx_t[NW - 1] if it == 0 else wc_t[NW - 1]
                warm(wsrc[:].rearrange("p a b -> p (a b)"), n=14)
                # squash gain in the log domain:
                #   gv = scale*sqrt(n2)/(1+n2)
                #      = exp(0.5*ln(n2) - ln(n2+1) + ln(scale))
                # -- 3 ACT ops + 1 tiny DVE stt; everything stays in the
                # natural_log_exp table set (no ~2.7us table swaps) and
                # the slow iterative DVE reciprocal disappears.
                lnn = sm.tile([128, 2 * CL], F32, tag="lnn")
                nc.scalar.activation(
                    lnn[:], n2[:].rearrange("p a b -> p (a b)"), AF.Ln,
                    scale=scale * scale,
                )
                lnd = sm.tile([128, 2 * CL], F32, tag="lnd")
                nc.scalar.activation(
                    lnd[:], n2[:].rearrange("p a b -> p (a b)"), AF.Ln,
                    bias=1.0,
                )
                arg = sm.tile([128, 2 * CL], F32, tag="arg")
                nc.vector.scalar_tensor_tensor(
                    arg[:], lnn[:], 0.5, lnd[:],
                    op0=ALU.mult, op1=ALU.subtract,
                )
                gv = sm.tile([128, 2, CL], F32, tag="gv")
                nc.scalar.activation(
                    gv[:].rearrange("p a b -> p (a b)"), arg[:], AF.Exp
                )
                for bt in range(2):
                    vdst = (vOut if last else vT)[:, bt, :]
                    nc.vector.tensor_tensor(
                        vdst.rearrange("p (c o) -> p c o", **co),
                        s_ps[bt].rearrange("p (c o) -> p c o", **co),
                        gv[:, bt, :].to_broadcast([128, CL, O]),
                        op=ALU.mult,
                    )
                    if last:
                        nc.sync.dma_start(
                            out_d[bt * 128:(bt + 1) * 128, :],
                            vOut[:, bt, :],
                        )
                if last:
                    break

                # ---- a-phase, transposed orientation ------------------
                # M'^T[k', (c,o)] = sum_b xb[b, k'] v[b, (c,o)] computed
                # in 30 k'-tiles of 128 (full 128 partitions under every
                # downstream per-element op, vs 96 in the M' orientation;
                # two [128,192] f32 tiles pack into ONE psum bank).
                # P^T = W2^T .* M'^T (ACT evicts psum->bf16, DVE 2x mult).
                # i-reduce: k' = i*192 + r is i-major, so j-blocks of 3
                # tiles (= 384 k' = exactly 2 i-slabs) are congruent
                # layouts -> a 4-op full-width DVE add tree collapses 10
                # blocks to 1, and 6 tiny selector matmuls (SELC maps
                # (p, jj) -> r) finish the cross-partition sum on the PE.
                # a_ij steers routing only, so everything here is bf16.
                pb2 = sm.tile([128, 30, M], BF16, tag="pb2")
                qT = sm.tile([KC, 2, R], BF16, tag="qT")
                cB = sm.tile([KC, 2, M], BF16, tag="cB")
                tb = sm.tile([128, 15, M], BF16, tag="tb")
                q2 = ps_q.tile([KC, 2, R], F32, tag="q2", name="q2")
                nmm = [0]

                def selc_mms(blocks):
                    # cross-partition finish on the PE: Q[(c,o), r]
                    # accumulated in psum over (block, jj); interleaves
                    # with later a-matmuls (different psum bank)
                    for bl in blocks:
                        for h in range(2):
                            for jj in range(3):
                                nc.tensor.matmul(
                                    q2[:, h, :],
                                    tb[:, 3 * bl + jj, h * KC:(h + 1) * KC],
                                    selc_t[:, jj, :],
                                    start=(nmm[0] == 0),
                                    stop=(nmm[0] == 29),
                                )
                                nmm[0] += 1

                with nc.allow_low_precision("a_ij steers routing only"):
                    for grp in range(8):          # tile groups of 4 (last: 2)
                        G = 4 if grp < 7 else 2
                        mps = ps_m.tile([128, 4, 256], F32, tag="mps")
                        for q in range(G):
                            j = 4 * grp + q
                            for bt in range(2):
                                nc.tensor.matmul(
                                    mps[:, q, 0:M],
                                    xb_t[j // 15][
                                        :, bt,
                                        (j % 15) * 128:(j % 15 + 1) * 128
                                    ],
                                    vT[:, bt, :],
                                    start=(bt == 0),
                                    stop=(bt == 1),
                                )
                        mb = sm.tile([128, 4, M], BF16, tag="mb")
                        nc.scalar.copy(mb[:, 0:G, :], mps[:, 0:G, 0:M])
                        # (NB: GpSimd offload tried and reverted -- ~3x
                        # slower AND contends with DVE for SBUF ports.)
                        nc.vector.tensor_tensor(
                            pb2[:, 4 * grp:4 * grp + G, :], mb[:, 0:G, :],
                            w2t_t[:, 4 * grp:4 * grp + G, :],
                            op=ALU.mult,
                        )
                        if grp == 4:
                            # j 0-4 and 15-19 evicted+multiplied: first
                            # level-1 fold, then block 0 can finish on PE
                            nc.vector.tensor_tensor(
                                tb[:, 0:5, :], pb2[:, 0:5, :],
                                pb2[:, 15:20, :], op=ALU.add,
                            )
                            selc_mms([0])
                        if grp == 6:
                            nc.vector.tensor_tensor(
                                tb[:, 5:10, :], pb2[:, 5:10, :],
                                pb2[:, 20:25, :], op=ALU.add,
                            )
                            selc_mms([1, 2])
                    nc.vector.tensor_tensor(
                        tb[:, 10:15, :], pb2[:, 10:15, :], pb2[:, 25:30, :],
                        op=ALU.add,
                    )
                    selc_mms([3, 4])
                    warm(tb[:].rearrange("p a b -> p (a b)"), n=12)
                with nc.allow_low_precision("a_ij steers routing only"):
                    nc.scalar.copy(qT[:], q2[:])
                # softmax over r, both halves stage-interleaved so ACT and
                # DVE overlap; no max-subtraction -- b_ij is a 2-step sum
                # of batch-mean agreements, bounded well inside exp's
                # f32 range
                eT = [sm.tile([CL // 2, R], F32, tag="eT", name=f"eT{g}")
                      for g in range(2)]
                ssum = [sm.tile([CL // 2, 1], F32, tag="ssum", name=f"ss{g}")
                        for g in range(2)]
                rs = [sm.tile([CL // 2, 1], F32, tag="rs", name=f"rs{g}")
                      for g in range(2)]
                cT = [sm.tile([CL // 2, R], F32, tag="cT", name=f"cT{g}")
                      for g in range(2)]
                for g in range(2):
                    # aps reuses the (already-evicted) q2 bank
                    aps = q2[0:CL // 2, g, :]
                    nc.tensor.matmul(
                        aps, sel[:], qT[:, g, :], start=True, stop=True
                    )
                    if it == 0:
                        nc.scalar.copy(bT[g][:], aps)
                    else:
                        nc.vector.tensor_tensor(
                            bT[g][:], bT[g][:], aps, op=ALU.add
                        )
                for g in range(2):
                    nc.scalar.activation(
                        eT[g][:], bT[g][:], AF.Exp, accum_out=ssum[g][:]
                    )
                # keep-warm through the softmax chain (anchored on the
                # just-finished qT half so it fires inside this stretch)
                warm(qT[:].rearrange("p a b -> p (a b)"), n=8)
                for g in range(2):
                    nc.vector.reciprocal(rs[g][:], ssum[g][:])
                for g in range(2):
                    nc.scalar.activation(
                        cT[g][:], eT[g][:], AF.Copy, scale=rs[g][:]
                    )
                # cB = transpose+o-broadcast of cT, done as one matmul per
                # (g, par): out[p, (c,o)] = cT.T @ REPL with
                # REPL[c, (c',o)] = delta(c,c') -- the broadcast rides the
                # matmul for free, and a single [96, 2, 96] evict per g
                # replaces four small broadcast copies.
                for g in range(2):
                    # cps also reuses the q2 bank (tile orders bank uses)
                    cps = q2[:, :, g * KC:(g + 1) * KC]
                    for par in range(2):
                        nc.tensor.matmul(
                            cps[:, par, :],
                            cT[g][:, par * KC:(par + 1) * KC],
                            repl_t[:],
                            start=True,
                            stop=True,
                        )
                    nc.scalar.copy(
                        cB[:, :, g * KC:(g + 1) * KC], cps
                    )
                # keep-warm through the cB-evict + wc stretch: emitted
                # after the cB matmuls (so they don't delay them in the
                # PE FIFO) but anchored on the already-complete cT
                warm(cB[:].rearrange("p a b -> p (a b)"), n=16)
                # wc = cB .* wt; chunk 0 per capsule-half (starts as soon
                # as that half's cB lands -> earlier first s-matmul),
                # chunks 1..4 full width (bf16 step-1 -> 2x).
                for g in range(2):
                    nc.vector.tensor_tensor(
                        wc_t[0][:, :, g * KC:(g + 1) * KC].rearrange(
                            "p (u par) m -> p u par m", par=2
                        ),
                        wx_t[0][:, :, g * KC:(g + 1) * KC].rearrange(
                            "p (u par) m -> p u par m", par=2
                        ),
                        cB[:, :, g * KC:(g + 1) * KC].unsqueeze(1)
                        .broadcast_to([KC, CPW // 2, 2, KC]),
                        op=ALU.mult,
                    )
                for j in range(1, NW):
                    nc.vector.tensor_tensor(
                        wc_t[j][:].rearrange(
                            "p (u par) m -> p u par m", par=2
                        ),
                        wx_t[j][:, :, 0:M].rearrange(
                            "p (u par) m -> p u par m", par=2
                        ),
                        cB[:].unsqueeze(1).broadcast_to(
                            [KC, CPW // 2, 2, M]
                        ),
                        op=ALU.mult,
                    )
    _split_multi_waits(nc)
    return nc


def prep_inputs(x, W, core):
    """Host-side shard prep for one core -> dict of bf16 arrays."""
    bf = ml_dtypes.bfloat16
    cs = core * CL
    # xt[p, t, b]: t = i*2 + par, r = par*96 + p   (k-layout A)
    xt = (
        np.transpose(x, (2, 1, 0))
        .reshape(I, 2, KC, B)
        .transpose(2, 0, 1, 3)
        .reshape(KC, NK, B)
    )
    # wt[p, t, c*16+o]  (k-layout A)
    Ws = W[:, cs:cs + CL]
    wt = (
        Ws.transpose(3, 0, 1, 2)
        .reshape(I, 2, KC, CL, O)
        .transpose(2, 0, 1, 3, 4)
        .reshape(KC, NK, M)
    )
    # xb[pb, bt, k']: k' = i*192 + r  (k-layout B, i-major so the i-reduce
    # runs as contiguous slab adds)
    xbi = np.ascontiguousarray(x.transpose(0, 2, 1)).reshape(B, KT)
    xb = xbi.reshape(2, 128, KT).transpose(1, 0, 2)
    # w2t[p, j, (c,o)]: transposed W2, k' = i*192 + r = j*128 + p
    w2flat = Ws.transpose(1, 2, 3, 0).reshape(M, KT)
    w2t = np.ascontiguousarray(w2flat.T).reshape(30, 128, M).transpose(1, 0, 2)
    # selc[p, jj, r] = 1 iff (jj*128 + p) % 192 == r  (block class -> r)
    selc = np.zeros((128, 3, R), dtype=ml_dtypes.bfloat16)
    for p in range(128):
        for jj in range(3):
            selc[p, jj, (jj * 128 + p) % R] = 1.0
    xt = xt.reshape(KC, NW, CPW, B).transpose(1, 0, 2, 3)
    wt = wt.reshape(KC, NW, CPW, M).transpose(1, 0, 2, 3)
    wx = np.concatenate([wt, xt], axis=-1)
    return {
        "wx": np.ascontiguousarray(wx).astype(bf),
        "xb": np.ascontiguousarray(xb).astype(bf),
        "w2t": np.ascontiguousarray(w2t).astype(bf),
        "selc": selc,
    }


_CACHED_NC = None


def kernel(x, W):
    from concourse.bass_utils import run_bass_kernel_spmd

    global _CACHED_NC
    x = np.asarray(x, dtype=np.float32)
    W = np.asarray(W, dtype=np.float32)
    if _CACHED_NC is None:
        _CACHED_NC = build_nc()
    nc = _CACHED_NC
    in_maps = [prep_inputs(x, W, core) for core in range(NCORES)]
    res = run_bass_kernel_spmd(nc, in_maps, list(range(NCORES)))
    v = np.empty((B, C, O), dtype=np.float32)
    for core in range(NCORES):
        v[:, core * CL:(core + 1) * CL, :] = (
            res.results[core]["out"].reshape(B, CL, O)
        )
    return v
